# revision 3
# baseline (speedup 1.0000x reference)
"""Trainium2 Bass kernel for nn_Decoder_72335839199969 (drug/protein
cross-attention decoder), data-parallel over batch across 8 NeuronCores.

Key algebraic shortcut: mean over p (and over l) commutes with the linear
map Wa, so the (B, Ld, Lp, D) einsum A = h @ Wa.T is never materialized.
Only the reduced tensors are needed:
    Hd[l, d] = sum_p relu(d_att[l, d] + p_att[p, d])   (B, Ld, D)
    Hp[p, d] = sum_l relu(d_att[l, d] + p_att[p, d])   (B, Lp, D)
then comp_att = sigmoid(Hd/Lp @ Wa.T + ba), prot_att = sigmoid(Hp/Ld @ Wa.T + ba).
This cuts ~77 GFLOP of matmul to ~1.5 GFLOP; the cost becomes the 25M-element
relu grid per core, split across engines:
  - ACT: relu(p_att + bias_col) with fused free-dim accumulate (-> Hd column)
  - DVE: scalar_tensor_tensor max(p_att + bias_col, 0) with fused accumulate
  - PE : accumulates Hp in PSUM via identity-matmul (bf16 h, fp32 accumulate);
         the d-tail (64 rows) is packed two-l-per-tile and folded by a
         [I64; I64] matrix so tail tiles cost the same as main tiles.
Layout: d on partitions (192 = 128 main + 64 tail), p on the free axis,
loop over l (128 iters) + 64 packed tail iters.
"""
import sys

sys.path.insert(0, "/opt/trn_rl_repo")

from contextlib import ExitStack

import ml_dtypes
import numpy as np

import concourse.bacc as bacc
import concourse.tile as tile
from concourse import mybir
from concourse.bass_utils import run_bass_kernel_spmd

F32 = mybir.dt.float32
BF16 = mybir.dt.bfloat16
AF = mybir.ActivationFunctionType
ALU = mybir.AluOpType

B, D, LD, LP = 8, 192, 128, 1024
DM, DT = 128, 64  # main/tail split of D
F1, F2, F3, NCLS = 1024, 1024, 512, 2
NCORES = 8

# Grid-phase engine split: n_act units on ScalarE (~1040ns each),
# the rest on VectorE (~690ns each). 192 units total.
N_UNITS = LD + LD // 2  # 128 main + 64 packed-tail
N_ACT = 76

_CACHE = {}


def _build():
    nc = bacc.Bacc("TRN2", target_bir_lowering=False, debug=False,
                   num_devices=NCORES)

    dram = {}

    def din(name, shape, dt=F32):
        dram[name] = nc.dram_tensor(name, list(shape), dt, kind="ExternalInput")
        return dram[name]

    dconv = din("dconv", [D, LD])
    pconv = din("pconv", [D, LP])
    wdt = din("wdt", [D, D])    # Wd.T  (d, e)
    wpt = din("wpt", [D, D])    # Wp.T
    wat = din("wat", [D, D])    # Wa.T
    bdm_d = din("bdm", [DM, 1])
    bdpk_d = din("bdpk", [DM, 1])   # [bd[128:]; bd[128:]]
    bpm_d = din("bpm", [DM, 1])
    bppk_d = din("bppk", [DM, 1])   # [bp[128:]; bp[128:]]
    bam_d = din("bam", [DM, 1])
    bat_d = din("bat", [DT, 1])
    w1t_d = din("w1t", [128, 3, F1])        # W1.T swizzled (kp, kc, m)
    w2t_d = din("w2t", [128, 8, F2])
    w3t_d = din("w3t", [128, 8, F3])
    wot_d = din("wot", [128, 4, NCLS])
    b1r_d = din("b1r", [1, F1])
    b2r_d = din("b2r", [1, F2])
    b3r_d = din("b3r", [1, F3])
    bor_d = din("bor", [1, NCLS])
    id_d = din("idbf", [128, 128], BF16)    # identity
    fold_d = din("foldbf", [128, DT], BF16)  # [I64; I64]
    shift_d = din("shiftm", [128, DT])       # [0; I64] fp32

    out_d = nc.dram_tensor("out", [NCLS], F32, kind="ExternalOutput")

    with tile.TileContext(nc) as tc, ExitStack() as ctx:
        const = ctx.enter_context(tc.tile_pool(name="const", bufs=1))
        hpool = ctx.enter_context(tc.tile_pool(name="hpool", bufs=4))
        ps_acc = ctx.enter_context(tc.tile_pool(name="ps_acc", bufs=1, space="PSUM"))
        ps_tmp = ctx.enter_context(tc.tile_pool(name="ps_tmp", bufs=1, space="PSUM"))

        # ---------------- phase 0: loads ----------------
        def load(name, shape, dt=F32, src=None):
            t = const.tile(shape, dt, name=f"sb_{name}")
            nc.sync.dma_start(out=t, in_=src if src is not None else dram[name].ap())
            return t

        # critical-path loads first
        pconv_m = load("pconv_m", [DM, LP], src=pconv[0:DM, :])
        pconv_t = load("pconv_t", [DT, LP], src=pconv[DM:D, :])
        dconv_m = load("dconv_m", [DM, LD], src=dconv[0:DM, :])
        dconv_t = load("dconv_t", [DT, LD], src=dconv[DM:D, :])
        wdt0 = load("wdt0", [DM, D], src=wdt[0:DM, :])
        wdt1 = load("wdt1", [DT, D], src=wdt[DM:D, :])
        wpt0 = load("wpt0", [DM, D], src=wpt[0:DM, :])
        wpt1 = load("wpt1", [DT, D], src=wpt[DM:D, :])
        wat0 = load("wat0", [DM, D], src=wat[0:DM, :])
        wat1 = load("wat1", [DT, D], src=wat[DM:D, :])
        bdm = load("bdm", [DM, 1])
        bdpk = load("bdpk", [DM, 1])
        bpm = load("bpm", [DM, 1])
        bppk = load("bppk", [DM, 1])
        bam = load("bam", [DM, 1])
        bat = load("bat", [DT, 1])
        idbf = load("idbf", [128, 128], BF16)
        foldbf = load("foldbf", [128, DT], BF16)
        shiftm = load("shiftm", [128, DT])
        # MLP weights (big, only needed at the very end)
        w1t = load("w1t", [128, 3, F1])
        w2t = load("w2t", [128, 8, F2])
        w3t = load("w3t", [128, 8, F3])
        wot = load("wot", [128, 4, NCLS])
        b1r = load("b1r", [1, F1])
        b2r = load("b2r", [1, F2])
        b3r = load("b3r", [1, F3])
        bor = load("bor", [1, NCLS])

        zeros_b = const.tile([DM, LP], BF16, name="zeros_b")
        nc.vector.memset(zeros_b, 0.0)
        ones_sb = const.tile([1, 1], F32, name="ones_sb")
        nc.vector.memset(ones_sb, 1.0)

        # ---------------- phase 1: d_att_T, p_att_T ----------------
        # d_att_T main: [e 0:128, l]
        ps_d = ps_tmp.tile([128, 128], F32, tag="ps_d", bufs=1, name="ps_d1")
        nc.tensor.matmul(ps_d, lhsT=wdt0[:, 0:DM], rhs=dconv_m, start=True, stop=False)
        nc.tensor.matmul(ps_d, lhsT=wdt1[:, 0:DM], rhs=dconv_t, start=False, stop=True)
        dattm = const.tile([DM, LD], F32, name="dattm")
        nc.scalar.activation(out=dattm, in_=ps_d, func=AF.Identity, bias=bdm, scale=1.0)

        # d_att_T tail, duplicated into both partition halves: [2x e 128:192, l]
        ps_dt = ps_tmp.tile([128, 128], F32, tag="ps_d", bufs=1, name="ps_d2")
        for half in (0, 1):
            o = ps_dt[half * DT:(half + 1) * DT, :]
            nc.tensor.matmul(o, lhsT=wdt0[:, DM:D], rhs=dconv_m, start=True, stop=False)
            nc.tensor.matmul(o, lhsT=wdt1[:, DM:D], rhs=dconv_t, start=False, stop=True)
        dattpk = const.tile([128, LD], F32, name="dattpk")
        nc.scalar.activation(out=dattpk, in_=ps_dt, func=AF.Identity, bias=bdpk, scale=1.0)

        # packed per-pair bias columns pb[:, j] = [datt_tail[:, 2j]; datt_tail[:, 2j+1]]
        pb = const.tile([128, LD // 2], F32, name="pb")
        top = dattpk[0:DT, :].rearrange("p (j two) -> p two j", two=2)
        bot = dattpk[DT:128, :].rearrange("p (j two) -> p two j", two=2)
        nc.vector.tensor_copy(pb[0:DT, :], top[:, 0, :])
        nc.vector.tensor_copy(pb[DT:128, :], bot[:, 1, :])

        # p_att_T main: [e 0:128, p]
        ps_p = ps_tmp.tile([128, LP], F32, tag="ps_p", bufs=1, name="ps_p1")
        for nh in (0, 1):
            o = ps_p[:, nh * 512:(nh + 1) * 512]
            nc.tensor.matmul(o, lhsT=wpt0[:, 0:DM], rhs=pconv_m[:, nh * 512:(nh + 1) * 512],
                             start=True, stop=False)
            nc.tensor.matmul(o, lhsT=wpt1[:, 0:DM], rhs=pconv_t[:, nh * 512:(nh + 1) * 512],
                             start=False, stop=True)
        pattm = const.tile([DM, LP], F32, name="pattm")
        nc.scalar.activation(out=pattm, in_=ps_p, func=AF.Identity, bias=bpm, scale=1.0)
        pattm_b = const.tile([DM, LP], BF16, name="pattm_b")
        nc.scalar.activation(out=pattm_b, in_=ps_p, func=AF.Identity, bias=bpm, scale=1.0)

        # p_att_T tail, duplicated: [2x e 128:192, p]
        ps_pp = ps_tmp.tile([128, LP], F32, tag="ps_p", bufs=1, name="ps_p2")
        for half in (0, 1):
            for nh in (0, 1):
                o = ps_pp[half * DT:(half + 1) * DT, nh * 512:(nh + 1) * 512]
                nc.tensor.matmul(o, lhsT=wpt0[:, DM:D], rhs=pconv_m[:, nh * 512:(nh + 1) * 512],
                                 start=True, stop=False)
                nc.tensor.matmul(o, lhsT=wpt1[:, DM:D], rhs=pconv_t[:, nh * 512:(nh + 1) * 512],
                                 start=False, stop=True)
        pattpk = const.tile([128, LP], F32, name="pattpk")
        nc.scalar.activation(out=pattpk, in_=ps_pp, func=AF.Identity, bias=bppk, scale=1.0)
        pattpk_b = const.tile([128, LP], BF16, name="pattpk_b")
        nc.scalar.activation(out=pattpk_b, in_=ps_pp, func=AF.Identity, bias=bppk, scale=1.0)

        # ---------------- phase 2: the relu grid ----------------
        HdTm = const.tile([DM, LD], F32, name="HdTm")      # sum_p h, main d rows
        HdP = const.tile([128, LD // 2], F32, name="HdP")  # packed tail accum cols
        hp_m = ps_acc.tile([128, LP], F32, name="hp_m")    # PSUM Hp main accum
        hp_p = ps_acc.tile([DT, LP], F32, name="hp_p")     # PSUM Hp tail (folded)

        units = []
        for j in range(LD // 2):
            units.append(("m", 2 * j))
            units.append(("m", 2 * j + 1))
            units.append(("p", j))

        n_m = 0
        n_p = 0
        for i, (kind, j) in enumerate(units):
            is_act = (i * N_ACT) // N_UNITS != ((i + 1) * N_ACT) // N_UNITS
            if kind == "m":
                in_f, in_b = pattm, pattm_b
                bias = dattm[:, j:j + 1]
                accum = HdTm[:, j:j + 1]
                acc_ps, lhs = hp_m, idbf
                first, last = n_m == 0, n_m == LD - 1
                n_m += 1
            else:
                in_f, in_b = pattpk, pattpk_b
                bias = pb[:, j:j + 1]
                accum = HdP[:, j:j + 1]
                acc_ps, lhs = hp_p, foldbf
                first, last = n_p == 0, n_p == LD // 2 - 1
                n_p += 1
            h = hpool.tile([128, LP], BF16, tag="ha" if is_act else "hd",
                           name=f"h_{i}")
            if is_act:
                nc.scalar.activation(out=h, in_=in_f, func=AF.Relu, bias=bias,
                                     scale=1.0, accum_out=accum)
            else:
                nc.vector.scalar_tensor_tensor(out=h, in0=in_b, scalar=bias,
                                               in1=zeros_b, op0=ALU.add,
                                               op1=ALU.max, accum_out=accum)
            for nh in (0, 1):
                nc.tensor.matmul(acc_ps[:, nh * 512:(nh + 1) * 512], lhsT=lhs,
                                 rhs=h[:, nh * 512:(nh + 1) * 512],
                                 start=first, stop=last)

        # ---------------- phase 3: attention application ----------------
        hpm_sb = const.tile([128, LP], F32, name="hpm_sb")
        nc.scalar.copy(hpm_sb, hp_m)
        hpt_sb = const.tile([DT, LP], F32, name="hpt_sb")
        nc.scalar.copy(hpt_sb, hp_p)

        # unpack HdP -> HdTt [64, 128]
        HdTt = const.tile([DT, LD], F32, name="HdTt")
        HdTt_v = HdTt.rearrange("p (j two) -> p two j", two=2)
        nc.vector.tensor_copy(HdTt_v[:, 0, :], HdP[0:DT, :])
        ps_sh = ps_tmp.tile([DT, LD // 2], F32, tag="ps_d", bufs=1, name="ps_sh")
        nc.tensor.matmul(ps_sh, lhsT=shiftm, rhs=HdP, start=True, stop=True)
        nc.vector.tensor_copy(HdTt_v[:, 1, :], ps_sh)

        # comp_att_T = sigmoid((Wa.T.T @ HdT)/LP + ba): [e, l]
        ps_ca = ps_tmp.tile([DM, LD], F32, tag="ps_d", bufs=1, name="ps_ca")
        nc.tensor.matmul(ps_ca, lhsT=wat0[:, 0:DM], rhs=HdTm, start=True, stop=False)
        nc.tensor.matmul(ps_ca, lhsT=wat1[:, 0:DM], rhs=HdTt, start=False, stop=True)
        cam = const.tile([DM, LD], F32, name="cam")
        nc.scalar.activation(out=cam, in_=ps_ca, func=AF.Sigmoid, bias=bam,
                             scale=1.0 / LP)
        ps_ct = ps_tmp.tile([DT, LD], F32, tag="ps_d", bufs=1, name="ps_ct")
        nc.tensor.matmul(ps_ct, lhsT=wat0[:, DM:D], rhs=HdTm, start=True, stop=False)
        nc.tensor.matmul(ps_ct, lhsT=wat1[:, DM:D], rhs=HdTt, start=False, stop=True)
        cat_ = const.tile([DT, LD], F32, name="cat_")
        nc.scalar.activation(out=cat_, in_=ps_ct, func=AF.Sigmoid, bias=bat,
                             scale=1.0 / LP)

        # prot_att_T = sigmoid((Wa.T.T @ Hp)/LD + ba): [e, p]
        ps_pa = ps_tmp.tile([DM, LP], F32, tag="ps_p", bufs=1, name="ps_pa")
        for nh in (0, 1):
            o = ps_pa[:, nh * 512:(nh + 1) * 512]
            nc.tensor.matmul(o, lhsT=wat0[:, 0:DM], rhs=hpm_sb[:, nh * 512:(nh + 1) * 512],
                             start=True, stop=False)
            nc.tensor.matmul(o, lhsT=wat1[:, 0:DM], rhs=hpt_sb[:, nh * 512:(nh + 1) * 512],
                             start=False, stop=True)
        pam = const.tile([DM, LP], F32, name="pam")
        nc.scalar.activation(out=pam, in_=ps_pa, func=AF.Sigmoid, bias=bam,
                             scale=1.0 / LD)
        ps_pt = ps_tmp.tile([DT, LP], F32, tag="ps_p", bufs=1, name="ps_pt")
        for nh in (0, 1):
            o = ps_pt[:, nh * 512:(nh + 1) * 512]
            nc.tensor.matmul(o, lhsT=wat0[:, DM:D], rhs=hpm_sb[:, nh * 512:(nh + 1) * 512],
                             start=True, stop=False)
            nc.tensor.matmul(o, lhsT=wat1[:, DM:D], rhs=hpt_sb[:, nh * 512:(nh + 1) * 512],
                             start=False, stop=True)
        pat = const.tile([DT, LP], F32, name="pat")
        nc.scalar.activation(out=pat, in_=ps_pt, func=AF.Sigmoid, bias=bat,
                             scale=1.0 / LD)

        # gated residual + max pool
        def gate_pool(att, conv, p, n, name):
            g = const.tile([p, n], F32, name=f"g_{name}")
            nc.vector.scalar_tensor_tensor(out=g, in0=att, scalar=0.5, in1=conv,
                                           op0=ALU.add, op1=ALU.mult)
            pool_t = const.tile([p, 1], F32, name=f"pool_{name}")
            nc.vector.tensor_reduce(pool_t, g, axis=mybir.AxisListType.X, op=ALU.max)
            return pool_t

        dpool_m = gate_pool(cam, dconv_m, DM, LD, "dm")
        dpool_t = gate_pool(cat_, dconv_t, DT, LD, "dt")
        ppool_m = gate_pool(pam, pconv_m, DM, LP, "pm")
        ppool_t = gate_pool(pat, pconv_t, DT, LP, "pt")

        # pair vector [384] as [128, 3] (chunk-major)
        pair_sb = const.tile([128, 3], F32, name="pair_sb")
        nc.vector.tensor_copy(pair_sb[:, 0:1], dpool_m)
        nc.vector.tensor_copy(pair_sb[0:DT, 1:2], dpool_t)
        nc.sync.dma_start(out=pair_sb[DT:128, 1:2], in_=ppool_m[0:DT, :])
        nc.sync.dma_start(out=pair_sb[0:DT, 2:3], in_=ppool_m[DT:128, :])
        nc.sync.dma_start(out=pair_sb[DT:128, 2:3], in_=ppool_t)

        # ---------------- phase 4: MLP ----------------
        def mlp_layer(prev, w_sb, brow, kc_n, m_n, name, act=True):
            ps = ps_tmp.tile([128, m_n], F32, tag="ps_mlp", bufs=1, name=f"psm_{name}")
            for m in range(m_n):
                o = ps[:, m:m + 1]
                for kc in range(kc_n):
                    nc.tensor.matmul(o, lhsT=w_sb[:, kc, m * 128:(m + 1) * 128],
                                     rhs=prev[:, kc:kc + 1],
                                     start=(kc == 0), stop=False)
                nc.tensor.matmul(o, lhsT=brow[0:1, m * 128:(m + 1) * 128],
                                 rhs=ones_sb, start=False, stop=True)
            o_sb = const.tile([128, m_n], F32, name=f"mlp_{name}")
            nc.scalar.activation(out=o_sb, in_=ps, func=AF.Lrelu, scale=1.0,
                                 alpha=0.01)
            return o_sb

        h1 = mlp_layer(pair_sb, w1t, b1r, 3, F1 // 128, "h1")
        h2 = mlp_layer(h1, w2t, b2r, F2 // 128, F2 // 128, "h2")
        h3 = mlp_layer(h2, w3t, b3r, F2 // 128, F3 // 128, "h3")

        ps_o = ps_tmp.tile([NCLS, 1], F32, tag="ps_mlp", bufs=1, name="ps_o")
        for kc in range(F3 // 128):
            nc.tensor.matmul(ps_o, lhsT=wot[:, kc, 0:NCLS], rhs=h3[:, kc:kc + 1],
                             start=(kc == 0), stop=False)
        nc.tensor.matmul(ps_o, lhsT=bor[0:1, 0:NCLS], rhs=ones_sb,
                         start=False, stop=True)
        out_sb = const.tile([NCLS, 1], F32, name="out_sb")
        nc.scalar.copy(out_sb, ps_o)
        nc.sync.dma_start(out=out_d[:], in_=out_sb)

    nc.compile()
    return nc


def prep_in_maps(drug_conv, protein_conv, Wd, bd, Wp, bp, Wa, ba,
                 W1, b1, W2, b2, W3, b3, Wo, bo):
    f = lambda a: np.ascontiguousarray(np.asarray(a, dtype=np.float32))
    bf = lambda a: np.ascontiguousarray(np.asarray(a).astype(ml_dtypes.bfloat16))
    drug_conv, protein_conv = f(drug_conv), f(protein_conv)

    def swz(WT, kc, m):   # (K, M) -> (128, kc, m)
        return f(WT.reshape(kc, 128, m).transpose(1, 0, 2))

    shared = {
        "wdt": f(np.asarray(Wd).T), "wpt": f(np.asarray(Wp).T),
        "wat": f(np.asarray(Wa).T),
        "bdm": f(bd[0:DM, None]), "bdpk": f(np.tile(np.asarray(bd)[DM:D], 2)[:, None]),
        "bpm": f(bp[0:DM, None]), "bppk": f(np.tile(np.asarray(bp)[DM:D], 2)[:, None]),
        "bam": f(ba[0:DM, None]), "bat": f(ba[DM:D, None]),
        "w1t": swz(f(np.asarray(W1).T), 3, F1),
        "w2t": swz(f(np.asarray(W2).T), 8, F2),
        "w3t": swz(f(np.asarray(W3).T), 8, F3),
        "wot": swz(f(np.asarray(Wo).T), 4, NCLS),
        "b1r": f(b1[None, :]), "b2r": f(b2[None, :]), "b3r": f(b3[None, :]),
        "bor": f(bo[None, :]),
        "idbf": bf(np.eye(128, dtype=np.float32)),
        "foldbf": bf(np.vstack([np.eye(DT, dtype=np.float32)] * 2)),
        "shiftm": f(np.vstack([np.zeros((DT, DT), np.float32),
                               np.eye(DT, dtype=np.float32)])),
    }
    return [
        {"dconv": drug_conv[b], "pconv": protein_conv[b], **shared}
        for b in range(NCORES)
    ]


def kernel(**inputs):
    if "nc" not in _CACHE:
        _CACHE["nc"] = _build()
    nc = _CACHE["nc"]
    in_maps = prep_in_maps(**inputs)
    res = run_bass_kernel_spmd(nc, in_maps, core_ids=list(range(NCORES)))
    _CACHE["last_results"] = res
    return np.stack([res.results[b]["out"] for b in range(NCORES)], axis=0)


# revision 4
# speedup vs baseline: 24.0024x; 24.0024x over previous
"""Trainium2 Bass kernel for nn_Decoder_72335839199969 (drug/protein
cross-attention decoder), data-parallel over batch across 8 NeuronCores.

Key algebraic shortcut: mean over p (and over l) commutes with the linear
map Wa, so the (B, Ld, Lp, D) einsum A = h @ Wa.T is never materialized.
Only the reduced tensors are needed:
    Hd[l, d] = sum_p relu(d_att[l, d] + p_att[p, d])   (B, Ld, D)
    Hp[p, d] = sum_l relu(d_att[l, d] + p_att[p, d])   (B, Lp, D)
then comp_att = sigmoid(Hd/Lp @ Wa.T + ba), prot_att = sigmoid(Hp/Ld @ Wa.T + ba).
This cuts ~77 GFLOP of matmul to ~1.5 GFLOP; the cost becomes the 25M-element
relu grid per core, split across engines:
  - ACT: relu(p_att + bias_col) with fused free-dim accumulate (-> Hd column)
  - DVE: scalar_tensor_tensor max(p_att + bias_col, 0) with fused accumulate
  - PE : accumulates Hp in PSUM via identity-matmul (bf16 h, fp32 accumulate);
         the d-tail (64 rows) is packed two-l-per-tile and folded by a
         [I64; I64] matrix so tail tiles cost the same as main tiles.
Layout: d on partitions (192 = 128 main + 64 tail), p on the free axis,
loop over l (128 iters) + 64 packed tail iters.
"""
import sys

sys.path.insert(0, "/opt/trn_rl_repo")

from contextlib import ExitStack

import ml_dtypes
import numpy as np

import concourse.bacc as bacc
import concourse.tile as tile
from concourse import mybir
from concourse.bass_utils import run_bass_kernel_spmd

F32 = mybir.dt.float32
BF16 = mybir.dt.bfloat16
AF = mybir.ActivationFunctionType
ALU = mybir.AluOpType

B, D, LD, LP = 8, 192, 128, 1024
DM, DT = 128, 64  # main/tail split of D
F1, F2, F3, NCLS = 1024, 1024, 512, 2
NCORES = 8

# Grid-phase engine split: n_act units on ScalarE (~1040ns each),
# the rest on VectorE (~690ns each). 192 units total.
N_UNITS = LD + LD // 2  # 128 main + 64 packed-tail
import os
N_ACT = int(os.environ.get("K_N_ACT", "76"))
K_SKIP_PE = os.environ.get("K_SKIP_PE", "0") == "1"

_CACHE = {}


def _build():
    nc = bacc.Bacc("TRN2", target_bir_lowering=False, debug=False,
                   num_devices=NCORES)

    dram = {}

    def din(name, shape, dt=F32):
        dram[name] = nc.dram_tensor(name, list(shape), dt, kind="ExternalInput")
        return dram[name]

    dconv = din("dconv", [D, LD])
    pconv = din("pconv", [D, LP])
    wdt = din("wdt", [D, D])    # Wd.T  (d, e)
    wpt = din("wpt", [D, D])    # Wp.T
    wat = din("wat", [D, D])    # Wa.T
    bdm_d = din("bdm", [DM, 1])
    bdpk_d = din("bdpk", [DM, 1])   # [bd[128:]; bd[128:]]
    bpm_d = din("bpm", [DM, 1])
    bppk_d = din("bppk", [DM, 1])   # [bp[128:]; bp[128:]]
    bam_d = din("bam", [DM, 1])
    bat_d = din("bat", [DT, 1])
    w1t_d = din("w1t", [128, 3, F1])        # W1.T swizzled (kp, kc, m)
    w2t_d = din("w2t", [128, 8, F2])
    w3t_d = din("w3t", [128, 8, F3])
    wot_d = din("wot", [128, 4, NCLS])
    b1r_d = din("b1r", [1, F1])
    b2r_d = din("b2r", [1, F2])
    b3r_d = din("b3r", [1, F3])
    bor_d = din("bor", [1, NCLS])
    id_d = din("idbf", [128, 128], BF16)    # identity
    fold_d = din("foldbf", [128, DT], BF16)  # [I64; I64]
    shift_d = din("shiftm", [128, DT])       # [0; I64] fp32

    out_d = nc.dram_tensor("out", [NCLS], F32, kind="ExternalOutput")

    with tile.TileContext(nc) as tc, ExitStack() as ctx:
        const = ctx.enter_context(tc.tile_pool(name="const", bufs=1))
        hpool = ctx.enter_context(tc.tile_pool(name="hpool", bufs=4))
        ps_acc = ctx.enter_context(tc.tile_pool(name="ps_acc", bufs=1, space="PSUM"))
        ps_tmp = ctx.enter_context(tc.tile_pool(name="ps_tmp", bufs=1, space="PSUM"))

        # ---------------- phase 0: loads ----------------
        def load(name, shape, dt=F32, src=None):
            t = const.tile(shape, dt, name=f"sb_{name}")
            nc.sync.dma_start(out=t, in_=src if src is not None else dram[name].ap())
            return t

        # critical-path loads first
        pconv_m = load("pconv_m", [DM, LP], src=pconv[0:DM, :])
        pconv_t = load("pconv_t", [DT, LP], src=pconv[DM:D, :])
        dconv_m = load("dconv_m", [DM, LD], src=dconv[0:DM, :])
        dconv_t = load("dconv_t", [DT, LD], src=dconv[DM:D, :])
        wdt0 = load("wdt0", [DM, D], src=wdt[0:DM, :])
        wdt1 = load("wdt1", [DT, D], src=wdt[DM:D, :])
        wpt0 = load("wpt0", [DM, D], src=wpt[0:DM, :])
        wpt1 = load("wpt1", [DT, D], src=wpt[DM:D, :])
        wat0 = load("wat0", [DM, D], src=wat[0:DM, :])
        wat1 = load("wat1", [DT, D], src=wat[DM:D, :])
        bdm = load("bdm", [DM, 1])
        bdpk = load("bdpk", [DM, 1])
        bpm = load("bpm", [DM, 1])
        bppk = load("bppk", [DM, 1])
        bam = load("bam", [DM, 1])
        bat = load("bat", [DT, 1])
        idbf = load("idbf", [128, 128], BF16)
        foldbf = load("foldbf", [128, DT], BF16)
        shiftm = load("shiftm", [128, DT])
        # MLP weights (big, only needed at the very end)
        w1t = load("w1t", [128, 3, F1])
        w2t = load("w2t", [128, 8, F2])
        w3t = load("w3t", [128, 8, F3])
        wot = load("wot", [128, 4, NCLS])
        b1r = load("b1r", [1, F1])
        b2r = load("b2r", [1, F2])
        b3r = load("b3r", [1, F3])
        bor = load("bor", [1, NCLS])

        zeros_b = const.tile([DM, LP], BF16, name="zeros_b")
        nc.vector.memset(zeros_b, 0.0)
        ones_sb = const.tile([1, 1], F32, name="ones_sb")
        nc.vector.memset(ones_sb, 1.0)

        # ---------------- phase 1: d_att_T, p_att_T ----------------
        # d_att_T main: [e 0:128, l]
        ps_d = ps_tmp.tile([128, 128], F32, tag="ps_d", bufs=1, name="ps_d1")
        nc.tensor.matmul(ps_d, lhsT=wdt0[:, 0:DM], rhs=dconv_m, start=True, stop=False)
        nc.tensor.matmul(ps_d, lhsT=wdt1[:, 0:DM], rhs=dconv_t, start=False, stop=True)
        dattm = const.tile([DM, LD], F32, name="dattm")
        nc.scalar.activation(out=dattm, in_=ps_d, func=AF.Identity, bias=bdm, scale=1.0)

        # d_att_T tail, duplicated into both partition halves: [2x e 128:192, l]
        ps_dt = ps_tmp.tile([128, 128], F32, tag="ps_d", bufs=1, name="ps_d2")
        for half in (0, 1):
            o = ps_dt[half * DT:(half + 1) * DT, :]
            nc.tensor.matmul(o, lhsT=wdt0[:, DM:D], rhs=dconv_m, start=True, stop=False)
            nc.tensor.matmul(o, lhsT=wdt1[:, DM:D], rhs=dconv_t, start=False, stop=True)
        dattpk = const.tile([128, LD], F32, name="dattpk")
        nc.scalar.activation(out=dattpk, in_=ps_dt, func=AF.Identity, bias=bdpk, scale=1.0)

        # packed per-pair bias columns pb[:, j] = [datt_tail[:, 2j]; datt_tail[:, 2j+1]]
        pb = const.tile([128, LD // 2], F32, name="pb")
        top = dattpk[0:DT, :].rearrange("p (j two) -> p two j", two=2)
        bot = dattpk[DT:128, :].rearrange("p (j two) -> p two j", two=2)
        nc.vector.tensor_copy(pb[0:DT, :], top[:, 0, :])
        nc.vector.tensor_copy(pb[DT:128, :], bot[:, 1, :])

        # p_att_T main: [e 0:128, p]
        ps_p = ps_tmp.tile([128, LP], F32, tag="ps_p", bufs=1, name="ps_p1")
        for nh in (0, 1):
            o = ps_p[:, nh * 512:(nh + 1) * 512]
            nc.tensor.matmul(o, lhsT=wpt0[:, 0:DM], rhs=pconv_m[:, nh * 512:(nh + 1) * 512],
                             start=True, stop=False)
            nc.tensor.matmul(o, lhsT=wpt1[:, 0:DM], rhs=pconv_t[:, nh * 512:(nh + 1) * 512],
                             start=False, stop=True)
        pattm = const.tile([DM, LP], F32, name="pattm")
        nc.scalar.activation(out=pattm, in_=ps_p, func=AF.Identity, bias=bpm, scale=1.0)
        pattm_b = const.tile([DM, LP], BF16, name="pattm_b")
        nc.scalar.activation(out=pattm_b, in_=ps_p, func=AF.Identity, bias=bpm, scale=1.0)

        # p_att_T tail, duplicated: [2x e 128:192, p]
        ps_pp = ps_tmp.tile([128, LP], F32, tag="ps_p", bufs=1, name="ps_p2")
        for half in (0, 1):
            for nh in (0, 1):
                o = ps_pp[half * DT:(half + 1) * DT, nh * 512:(nh + 1) * 512]
                nc.tensor.matmul(o, lhsT=wpt0[:, DM:D], rhs=pconv_m[:, nh * 512:(nh + 1) * 512],
                                 start=True, stop=False)
                nc.tensor.matmul(o, lhsT=wpt1[:, DM:D], rhs=pconv_t[:, nh * 512:(nh + 1) * 512],
                                 start=False, stop=True)
        pattpk = const.tile([128, LP], F32, name="pattpk")
        nc.scalar.activation(out=pattpk, in_=ps_pp, func=AF.Identity, bias=bppk, scale=1.0)
        pattpk_b = const.tile([128, LP], BF16, name="pattpk_b")
        nc.scalar.activation(out=pattpk_b, in_=ps_pp, func=AF.Identity, bias=bppk, scale=1.0)

        # ---------------- phase 2: the relu grid ----------------
        HdTm = const.tile([DM, LD], F32, name="HdTm")      # sum_p h, main d rows
        HdP = const.tile([128, LD // 2], F32, name="HdP")  # packed tail accum cols
        hp_m = ps_acc.tile([128, LP], F32, name="hp_m")    # PSUM Hp main accum
        hp_p = ps_acc.tile([DT, LP], F32, name="hp_p")     # PSUM Hp tail (folded)

        units = []
        for j in range(LD // 2):
            units.append(("m", 2 * j))
            units.append(("m", 2 * j + 1))
            units.append(("p", j))

        n_m = 0
        n_p = 0
        for i, (kind, j) in enumerate(units):
            is_act = (i * N_ACT) // N_UNITS != ((i + 1) * N_ACT) // N_UNITS
            if kind == "m":
                in_f, in_b = pattm, pattm_b
                bias = dattm[:, j:j + 1]
                accum = HdTm[:, j:j + 1]
                acc_ps, lhs = hp_m, idbf
                first, last = n_m == 0, n_m == LD - 1
                n_m += 1
            else:
                in_f, in_b = pattpk, pattpk_b
                bias = pb[:, j:j + 1]
                accum = HdP[:, j:j + 1]
                acc_ps, lhs = hp_p, foldbf
                first, last = n_p == 0, n_p == LD // 2 - 1
                n_p += 1
            h = hpool.tile([128, LP], BF16, tag="ha" if is_act else "hd",
                           name=f"h_{i}")
            if is_act:
                nc.scalar.activation(out=h, in_=in_f, func=AF.Relu, bias=bias,
                                     scale=1.0, accum_out=accum)
            else:
                nc.vector.scalar_tensor_tensor(out=h, in0=in_b, scalar=bias,
                                               in1=zeros_b, op0=ALU.add,
                                               op1=ALU.max, accum_out=accum)
            if not K_SKIP_PE:
                for nh in (0, 1):
                    nc.tensor.matmul(acc_ps[:, nh * 512:(nh + 1) * 512], lhsT=lhs,
                                     rhs=h[:, nh * 512:(nh + 1) * 512],
                                     start=first, stop=last)

        # ---------------- phase 3: attention application ----------------
        hpm_sb = const.tile([128, LP], F32, name="hpm_sb")
        nc.scalar.copy(hpm_sb, hp_m)
        hpt_sb = const.tile([DT, LP], F32, name="hpt_sb")
        nc.scalar.copy(hpt_sb, hp_p)

        # unpack HdP -> HdTt [64, 128]
        HdTt = const.tile([DT, LD], F32, name="HdTt")
        HdTt_v = HdTt.rearrange("p (j two) -> p two j", two=2)
        nc.vector.tensor_copy(HdTt_v[:, 0, :], HdP[0:DT, :])
        ps_sh = ps_tmp.tile([DT, LD // 2], F32, tag="ps_d", bufs=1, name="ps_sh")
        nc.tensor.matmul(ps_sh, lhsT=shiftm, rhs=HdP, start=True, stop=True)
        nc.vector.tensor_copy(HdTt_v[:, 1, :], ps_sh)

        # comp_att_T = sigmoid((Wa.T.T @ HdT)/LP + ba): [e, l]
        ps_ca = ps_tmp.tile([DM, LD], F32, tag="ps_d", bufs=1, name="ps_ca")
        nc.tensor.matmul(ps_ca, lhsT=wat0[:, 0:DM], rhs=HdTm, start=True, stop=False)
        nc.tensor.matmul(ps_ca, lhsT=wat1[:, 0:DM], rhs=HdTt, start=False, stop=True)
        cam = const.tile([DM, LD], F32, name="cam")
        nc.scalar.activation(out=cam, in_=ps_ca, func=AF.Sigmoid, bias=bam,
                             scale=1.0 / LP)
        ps_ct = ps_tmp.tile([DT, LD], F32, tag="ps_d", bufs=1, name="ps_ct")
        nc.tensor.matmul(ps_ct, lhsT=wat0[:, DM:D], rhs=HdTm, start=True, stop=False)
        nc.tensor.matmul(ps_ct, lhsT=wat1[:, DM:D], rhs=HdTt, start=False, stop=True)
        cat_ = const.tile([DT, LD], F32, name="cat_")
        nc.scalar.activation(out=cat_, in_=ps_ct, func=AF.Sigmoid, bias=bat,
                             scale=1.0 / LP)

        # prot_att_T = sigmoid((Wa.T.T @ Hp)/LD + ba): [e, p]
        ps_pa = ps_tmp.tile([DM, LP], F32, tag="ps_p", bufs=1, name="ps_pa")
        for nh in (0, 1):
            o = ps_pa[:, nh * 512:(nh + 1) * 512]
            nc.tensor.matmul(o, lhsT=wat0[:, 0:DM], rhs=hpm_sb[:, nh * 512:(nh + 1) * 512],
                             start=True, stop=False)
            nc.tensor.matmul(o, lhsT=wat1[:, 0:DM], rhs=hpt_sb[:, nh * 512:(nh + 1) * 512],
                             start=False, stop=True)
        pam = const.tile([DM, LP], F32, name="pam")
        nc.scalar.activation(out=pam, in_=ps_pa, func=AF.Sigmoid, bias=bam,
                             scale=1.0 / LD)
        ps_pt = ps_tmp.tile([DT, LP], F32, tag="ps_p", bufs=1, name="ps_pt")
        for nh in (0, 1):
            o = ps_pt[:, nh * 512:(nh + 1) * 512]
            nc.tensor.matmul(o, lhsT=wat0[:, DM:D], rhs=hpm_sb[:, nh * 512:(nh + 1) * 512],
                             start=True, stop=False)
            nc.tensor.matmul(o, lhsT=wat1[:, DM:D], rhs=hpt_sb[:, nh * 512:(nh + 1) * 512],
                             start=False, stop=True)
        pat = const.tile([DT, LP], F32, name="pat")
        nc.scalar.activation(out=pat, in_=ps_pt, func=AF.Sigmoid, bias=bat,
                             scale=1.0 / LD)

        # gated residual + max pool
        def gate_pool(att, conv, p, n, name):
            g = const.tile([p, n], F32, name=f"g_{name}")
            nc.vector.scalar_tensor_tensor(out=g, in0=att, scalar=0.5, in1=conv,
                                           op0=ALU.add, op1=ALU.mult)
            pool_t = const.tile([p, 1], F32, name=f"pool_{name}")
            nc.vector.tensor_reduce(pool_t, g, axis=mybir.AxisListType.X, op=ALU.max)
            return pool_t

        dpool_m = gate_pool(cam, dconv_m, DM, LD, "dm")
        dpool_t = gate_pool(cat_, dconv_t, DT, LD, "dt")
        ppool_m = gate_pool(pam, pconv_m, DM, LP, "pm")
        ppool_t = gate_pool(pat, pconv_t, DT, LP, "pt")

        # pair vector [384] as [128, 3] (chunk-major)
        pair_sb = const.tile([128, 3], F32, name="pair_sb")
        nc.vector.tensor_copy(pair_sb[:, 0:1], dpool_m)
        nc.vector.tensor_copy(pair_sb[0:DT, 1:2], dpool_t)
        nc.sync.dma_start(out=pair_sb[DT:128, 1:2], in_=ppool_m[0:DT, :])
        nc.sync.dma_start(out=pair_sb[0:DT, 2:3], in_=ppool_m[DT:128, :])
        nc.sync.dma_start(out=pair_sb[DT:128, 2:3], in_=ppool_t)

        # ---------------- phase 4: MLP ----------------
        def mlp_layer(prev, w_sb, brow, kc_n, m_n, name, act=True):
            ps = ps_tmp.tile([128, m_n], F32, tag="ps_mlp", bufs=1, name=f"psm_{name}")
            for m in range(m_n):
                o = ps[:, m:m + 1]
                for kc in range(kc_n):
                    nc.tensor.matmul(o, lhsT=w_sb[:, kc, m * 128:(m + 1) * 128],
                                     rhs=prev[:, kc:kc + 1],
                                     start=(kc == 0), stop=False)
                nc.tensor.matmul(o, lhsT=brow[0:1, m * 128:(m + 1) * 128],
                                 rhs=ones_sb, start=False, stop=True)
            o_sb = const.tile([128, m_n], F32, name=f"mlp_{name}")
            nc.scalar.activation(out=o_sb, in_=ps, func=AF.Lrelu, scale=1.0,
                                 alpha=0.01)
            return o_sb

        h1 = mlp_layer(pair_sb, w1t, b1r, 3, F1 // 128, "h1")
        h2 = mlp_layer(h1, w2t, b2r, F2 // 128, F2 // 128, "h2")
        h3 = mlp_layer(h2, w3t, b3r, F2 // 128, F3 // 128, "h3")

        ps_o = ps_tmp.tile([NCLS, 1], F32, tag="ps_mlp", bufs=1, name="ps_o")
        for kc in range(F3 // 128):
            nc.tensor.matmul(ps_o, lhsT=wot[:, kc, 0:NCLS], rhs=h3[:, kc:kc + 1],
                             start=(kc == 0), stop=False)
        nc.tensor.matmul(ps_o, lhsT=bor[0:1, 0:NCLS], rhs=ones_sb,
                         start=False, stop=True)
        out_sb = const.tile([NCLS, 1], F32, name="out_sb")
        nc.scalar.copy(out_sb, ps_o)
        nc.sync.dma_start(out=out_d[:], in_=out_sb)

    nc.compile()
    return nc


def prep_in_maps(drug_conv, protein_conv, Wd, bd, Wp, bp, Wa, ba,
                 W1, b1, W2, b2, W3, b3, Wo, bo):
    f = lambda a: np.ascontiguousarray(np.asarray(a, dtype=np.float32))
    bf = lambda a: np.ascontiguousarray(np.asarray(a).astype(ml_dtypes.bfloat16))
    drug_conv, protein_conv = f(drug_conv), f(protein_conv)

    def swz(WT, kc, m):   # (K, M) -> (128, kc, m)
        return f(WT.reshape(kc, 128, m).transpose(1, 0, 2))

    shared = {
        "wdt": f(np.asarray(Wd).T), "wpt": f(np.asarray(Wp).T),
        "wat": f(np.asarray(Wa).T),
        "bdm": f(bd[0:DM, None]), "bdpk": f(np.tile(np.asarray(bd)[DM:D], 2)[:, None]),
        "bpm": f(bp[0:DM, None]), "bppk": f(np.tile(np.asarray(bp)[DM:D], 2)[:, None]),
        "bam": f(ba[0:DM, None]), "bat": f(ba[DM:D, None]),
        "w1t": swz(f(np.asarray(W1).T), 3, F1),
        "w2t": swz(f(np.asarray(W2).T), 8, F2),
        "w3t": swz(f(np.asarray(W3).T), 8, F3),
        "wot": swz(f(np.asarray(Wo).T), 4, NCLS),
        "b1r": f(b1[None, :]), "b2r": f(b2[None, :]), "b3r": f(b3[None, :]),
        "bor": f(bo[None, :]),
        "idbf": bf(np.eye(128, dtype=np.float32)),
        "foldbf": bf(np.vstack([np.eye(DT, dtype=np.float32)] * 2)),
        "shiftm": f(np.vstack([np.zeros((DT, DT), np.float32),
                               np.eye(DT, dtype=np.float32)])),
    }
    return [
        {"dconv": drug_conv[b], "pconv": protein_conv[b], **shared}
        for b in range(NCORES)
    ]


def kernel(**inputs):
    if "nc" not in _CACHE:
        _CACHE["nc"] = _build()
    nc = _CACHE["nc"]
    in_maps = prep_in_maps(**inputs)
    res = run_bass_kernel_spmd(nc, in_maps, core_ids=list(range(NCORES)))
    _CACHE["last_results"] = res
    return np.stack([res.results[b]["out"] for b in range(NCORES)], axis=0)
